# revision 1
# baseline (speedup 1.0000x reference)
"""Trainium2 Bass kernel: EnhancedVariancePooling (v5 edge-split).

Same algorithm as v3 (pairwise prefix scans + strided prefix
differences), but the first and last row-tiles stream their 3990-sample
T-axis as two chained halves (split loads, split squares, chained
scans), shortening the pipeline fill and drain by roughly half a tile's
serial chain. Window extraction is unchanged except the single-sample
correction gathers split at the half boundary.
"""

import numpy as np

import concourse.bacc as bacc
import concourse.bass as bass
import concourse.tile as tile
import concourse.mybir as mybir
from concourse.bass_utils import run_bass_kernel_spmd

B, C, T = 64, 128, 4000
KS, ST = 75, 15
O = (T - KS) // ST + 1          # 262
NCH = O + (KS // ST) - 1        # 266
TU = NCH * ST                   # 3990
NPAIR = TU // 2                 # 1995
HALF = 1996                     # first-half samples (even; 998 pairs)
VAR_MIN, VAR_MAX = 1e-6, 1e6

N_CORES = 8
B_PER = B // N_CORES
ROWS = B_PER * C                # 1024
P = 128
NTILES = ROWS // P              # 8

F32 = mybir.dt.float32
ALU = mybir.AluOpType
ACTF = mybir.ActivationFunctionType

_NC = None


def _build():
    nc = bacc.Bacc()
    x = nc.declare_dram_parameter("x", [ROWS, T], F32, isOutput=False)
    y = nc.declare_dram_parameter("y", [ROWS, O], F32, isOutput=True)

    NW = O // 2  # 131 windows per parity

    with tile.TileContext(nc) as tc:
        with (
            tc.tile_pool(name="big", bufs=4) as big,
            tc.tile_pool(name="half", bufs=2) as halfp,
            tc.tile_pool(name="sq", bufs=3) as sqp,
            tc.tile_pool(name="sqh", bufs=2) as sqhp,
            tc.tile_pool(name="pfx", bufs=2) as pfx,
            tc.tile_pool(name="small", bufs=2) as small,
            tc.tile_pool(name="out", bufs=8) as outp,
        ):

            def emit_front(it, split):
                """Load + square + prefix scans.  Returns
                (parts, p2x, p2q) where parts is a list of
                (x_tile, xq_tile, seg_start, seg_len)."""
                r0 = it * P
                p2x = pfx.tile([P, NPAIR + 1], F32, tag="p2x")
                nc.gpsimd.memset(p2x[:, 0:1], 0.0)
                p2q = pfx.tile([P, NPAIR + 1], F32, tag="p2q")
                nc.gpsimd.memset(p2q[:, 0:1], 0.0)

                if not split:
                    xt = big.tile([P, TU], F32, tag="xt")
                    nc.sync.dma_start(out=xt, in_=x[r0 : r0 + P, 0:TU])
                    xq = sqp.tile([P, TU], F32, tag="xq")
                    nc.scalar.activation(xq, xt, ACTF.Square)
                    nc.vector.tensor_tensor_scan(
                        p2x[:, 1:], xt[:, 0:TU:2], xt[:, 1:TU:2],
                        initial=0.0, op0=ALU.add, op1=ALU.add,
                    )
                    nc.vector.tensor_tensor_scan(
                        p2q[:, 1:], xq[:, 0:TU:2], xq[:, 1:TU:2],
                        initial=0.0, op0=ALU.add, op1=ALU.add,
                    )
                    return [(xt, xq, 0, TU)], p2x, p2q

                parts = []
                segs = [(0, HALF), (HALF, TU - HALF)]
                for (s, ln) in segs:
                    xh = halfp.tile([P, HALF], F32, tag="xh")
                    nc.sync.dma_start(
                        out=xh[:, :ln], in_=x[r0 : r0 + P, s : s + ln]
                    )
                    qh = sqhp.tile([P, HALF], F32, tag="qh")
                    nc.scalar.activation(qh[:, :ln], xh[:, :ln], ACTF.Square)
                    j0, j1 = s // 2, (s + ln) // 2
                    init_x = 0.0 if s == 0 else p2x[:, j0 : j0 + 1]
                    init_q = 0.0 if s == 0 else p2q[:, j0 : j0 + 1]
                    nc.vector.tensor_tensor_scan(
                        p2x[:, j0 + 1 : j1 + 1],
                        xh[:, 0:ln:2], xh[:, 1:ln:2],
                        initial=init_x, op0=ALU.add, op1=ALU.add,
                    )
                    nc.vector.tensor_tensor_scan(
                        p2q[:, j0 + 1 : j1 + 1],
                        qh[:, 0:ln:2], qh[:, 1:ln:2],
                        initial=init_q, op0=ALU.add, op1=ALU.add,
                    )
                    parts.append((xh, qh, s, ln))
                return parts, p2x, p2q

            def corrections(so, a, w0, x_off, sign, parts, which, eng=None):
                """so[w0+2v] = a[v] + sign*x[x_off+30v], v in [0, NW), with
                the gather split across `parts`.  which: 0 -> x, 1 -> xq."""
                eng = eng or nc.gpsimd
                for (xh, qh, s, ln) in parts:
                    xv = (xh, qh)[which]
                    # v range whose gather index falls in [s, s+ln)
                    v0 = max(0, -(-(s - x_off) // 30))          # ceil
                    v1 = min(NW, (s + ln - 1 - x_off) // 30 + 1)
                    if v1 <= v0:
                        continue
                    n = v1 - v0
                    off = x_off + 30 * v0 - s
                    eng.tensor_tensor(
                        out=so[:, w0 + 2 * v0 : w0 + 2 * (v1 - 1) + 1 : 2],
                        in0=a[:, v0:v1],
                        in1=xv[:, off : off + 30 * (n - 1) + 1 : 30],
                        op=ALU.subtract if sign < 0 else ALU.add,
                    )

            def emit_epilogue(state):
                it, (parts, p2x, p2q) = state
                r0 = it * P
                s1 = small.tile([P, O], F32, tag="s1")
                s2 = small.tile([P, O], F32, tag="s2")
                # groups: (w0, m0, dd, x_off, sign)
                for (w0, m0, dd, x_off, sign) in (
                    (0, 0, 0, 75, -1),   # even w: P2[15u+38]-P2[15u]  -x[30u+75]
                    (1, 7, 1, 15, +1),   # odd  w: P2[15u+45]-P2[15u+8]+x[30u+15]
                ):
                    last = it == NTILES - 1
                    for p2, which, so in ((p2x, 0, s1), (p2q, 1, s2)):
                        eng = nc.vector if (last and which == 1) else nc.gpsimd
                        a = small.tile([P, NW], F32, tag="pd")
                        eng.tensor_tensor(
                            out=a,
                            in0=p2[:, m0 + 38 : m0 + 38 + 15 * (NW - 1) + 1 : 15],
                            in1=p2[:, m0 + dd : m0 + dd + 15 * (NW - 1) + 1 : 15],
                            op=ALU.subtract,
                        )
                        corrections(so, a, w0, x_off, sign, parts, which, eng)

                # wv = S1^2/75 - S2  (= -74*var)
                ss = small.tile([P, O], F32, tag="ss")
                nc.scalar.activation(ss, s1, ACTF.Square)
                wv = small.tile([P, O], F32, tag="wv")
                nc.vector.scalar_tensor_tensor(
                    out=wv, in0=ss, scalar=1.0 / KS, in1=s2,
                    op0=ALU.mult, op1=ALU.subtract,
                )
                wc = small.tile([P, O], F32, tag="wc")
                nc.vector.tensor_scalar(
                    out=wc, in0=wv,
                    scalar1=-(KS - 1.0) * VAR_MAX, scalar2=-(KS - 1.0) * VAR_MIN,
                    op0=ALU.max, op1=ALU.min,
                )
                ot = outp.tile([P, O], F32, tag="ot")
                nc.scalar.activation(ot, wc, ACTF.Ln, scale=-1.0 / (KS - 1.0))
                deferred_stores.append((r0, ot))

            deferred_stores = []
            prev = None
            for it in range(NTILES):
                split = it == NTILES - 1
                cur = (it, emit_front(it, split))
                if prev is not None:
                    emit_epilogue(prev)
                prev = cur
            emit_epilogue(prev)
            # stores last on the SP ring: FIFO order keeps them from
            # stealing SDMA bandwidth from the input stream.
            for r0, ot in deferred_stores:
                nc.sync.dma_start(out=y[r0 : r0 + P, :], in_=ot)
    nc.compile()
    return nc


def _get_nc():
    global _NC
    if _NC is None:
        _NC = _build()
    return _NC


_RUNNER = None


def _get_runner():
    """Build the sharded PJRT callable once (run_bass_via_pjrt re-traces
    jax on every call; caching the jitted function makes repeat kernel()
    calls cheap)."""
    global _RUNNER
    if _RUNNER is not None:
        return _RUNNER

    import jax
    from jax.sharding import Mesh, PartitionSpec
    from jax.experimental.shard_map import shard_map
    from concourse import bass2jax

    nc = _get_nc()
    bass2jax.install_neuronx_cc_hook()
    partition_name = nc.partition_id_tensor.name if nc.partition_id_tensor else None

    def _body(xin, yzero):
        operands = [xin, yzero]
        if partition_name is not None:
            operands.append(bass2jax.partition_id_tensor())
        outs = bass2jax._bass_exec_p.bind(
            *operands,
            out_avals=(jax.core.ShapedArray((ROWS, O), np.float32),),
            in_names=("x", "y") + (() if partition_name is None else (partition_name,)),
            out_names=("y",),
            lowering_input_output_aliases=(),
            sim_require_finite=True,
            sim_require_nnan=True,
            nc=nc,
        )
        return tuple(outs)

    devices = jax.devices()[:N_CORES]
    mesh = Mesh(np.asarray(devices), ("core",))
    sharded = jax.jit(
        shard_map(
            _body, mesh=mesh,
            in_specs=(PartitionSpec("core"), PartitionSpec("core")),
            out_specs=(PartitionSpec("core"),),
            check_rep=False,
        ),
        donate_argnums=(1,),
        keep_unused=True,
    )
    _RUNNER = sharded
    return sharded


def kernel(x: np.ndarray) -> np.ndarray:
    x = np.ascontiguousarray(np.asarray(x), dtype=np.float32)
    assert x.shape == (B, C, T)
    flat = x.reshape(N_CORES * ROWS, T)
    try:
        runner = _get_runner()
        (out,) = runner(flat, np.zeros((N_CORES * ROWS, O), np.float32))
        return np.asarray(out).reshape(B, C, O)
    except Exception:
        # Fallback: the supported (but per-call re-tracing) path.
        nc = _get_nc()
        xs = x.reshape(N_CORES, ROWS, T)
        in_maps = [{"x": xs[i]} for i in range(N_CORES)]
        res = run_bass_kernel_spmd(nc, in_maps, list(range(N_CORES)))
        out = np.stack([res.results[i]["y"] for i in range(N_CORES)])
        return out.reshape(B, C, O)

